# revision 119
# baseline (speedup 1.0000x reference)
"""BoxOnlyHungarianMatcher cost matrix on 8 TRN2 NeuronCores.

cost[i,j] = 5*L1(pred_i, gt_j) + 2*(-GIoU(pred_i, gt_j))
pred: [16,900,4] cxcywh, gt: [1600,4] cxcywh -> out [16,900,1600] f32.

Sharding: data-parallel over flattened pred rows (14400 = 8 * 1800).
Each core: 14 full blocks of 128 preds, plus one repacked tail pass
(last 8 preds x 1600 gts as [128 = 8 preds x 16 slices, 100 gt]).

Per block (partitions = 128 preds, free dim = 1600 gts):
  DVE custom ops (f16 in, f16 out):
    t_x  = min(gx1, px1) - max(gx0, px0)          [TX_MINMAX]
    t_y  = min(gy1, py1) - max(gy0, py0)
    inter = relu(t_x)*relu(t_y)                   [RELU_MUL]
  DVE stock f16 (ts 4x / tt 2x):
    gwpw = gw+pw ; ghph = gh+ph ; eh = ghph-t_y ; s16 = garea+parea
    A' = inter*ru16 (= iou/16) ; B' = union*re16 (= q/16)
  Pool (gpsimd):
    ew = gwpw - t_x ; union = s16 - inter ; earea = ew * eh
  ACT:
    ru16|re16 = Reciprocal(16*union | 16*earea)  (one paired pass)
    a1..a4 = |5*gc - 5*pc|  (Abs, scale=5, per-partition bias)
  PE:  psum = a1+a2+a3+a4 + (-32)*(A'+B')  (identity-matmul accumulate
       into 2-bank psum tiles)
  final: cost = psum + 2.0 evacuated in one instruction per psum tile
       (DVE / ACT alternating) -> f16 SBUF, one DMA out per block.
"""

import numpy as np

import concourse.bass as bass
import concourse.bacc as bacc
import concourse.tile as tile
from concourse import mybir
from concourse.bass_utils import run_bass_kernel_spmd

F32 = mybir.dt.float32
F16 = mybir.dt.float16

B, Q, M = 16, 900, 1600
N = B * Q            # 14400
NCORES = 8
QSH = N // NCORES    # 1800 preds per core
NB = 14              # full blocks of 128 preds
MT = M // 16         # 100: tail gt-slice width (8 preds x 16 slices = 128)

SR = 16.0            # reciprocal pre-scale: ru16 = 1/(SR*union)
BPP = 0              # columns of Bp computed on Pool (rest on DVE)
EHP = 0              # columns of eh computed on Pool (rest on DVE)
ABS_DVE = 0          # columns of the |5gw-5pw| term computed on DVE

# per-partition pred feature rows (f32), laid out [128, NPF, NB]
PF_PX0, PF_PX1, PF_PY0, PF_PY1, PF_PW, PF_PH, PF_PA, PF_B5CX, PF_B5CY, PF_B5W, PF_B5H = range(11)
NPF = 11

# g feature row order in gfeat / gtail (all f16); x0 = x1 - w is derived
# in-op by TX_MINMAX2, so corner-low rows are never materialized
GR_X1, GR_W, GR_Y1, GR_H, GR_CX, GR_CY, GR_A = range(7)
NGR = 7

_CUSTOM_REGISTERED = False
_TX_MINMAX = None
_RELU_MUL = None


def _register_custom_ops():
    """Append our fused DVE ops to the concourse custom-op table (rows 17+ are free)."""
    global _CUSTOM_REGISTERED, _TX_MINMAX, _RELU_MUL
    if _CUSTOM_REGISTERED:
        return
    from concourse import dve_ops
    from concourse.dve_ops import DveOp, OPS, _SUB_OPCODE_FOR_NAME
    from concourse.dve_spec import (
        Spec, Src0, Src1, C0, C1, lower, maxx, minn, relu, _has_src1,
    )
    from concourse.dve_uop import DveOpSpec

    def _register(name, spec):
        if name in _SUB_OPCODE_FOR_NAME:
            for op in OPS:
                if op.name == name:
                    return op
            raise RuntimeError(f"row taken but op {name} not in OPS")
        op = DveOp(name, spec, subdim=False, uops_sha={})
        row = max(_SUB_OPCODE_FOR_NAME.values()) + 1
        assert row < 0x20, "out of custom-DVE rows"
        _SUB_OPCODE_FOR_NAME[name] = row
        for ver in ("v3",):  # TRN2
            compiled = DveOpSpec(
                name=name, opcode=row, uops=lower(spec, ver=ver),
                rd1_en=_has_src1(spec),
            )
            op.uops_sha[ver] = compiled.sha(ver)
        OPS.append(op)
        dve_ops.CUSTOM_DVE_SPECS[name] = spec
        return op

    # t = min(hi, p_hi) - max(hi - w, p_lo): the low corner is derived from
    # the high corner and the width inside the op (saves two g-row loads)
    _TX_MINMAX = _register(
        "ANT_TX_MINMAX2",
        Spec(
            body=minn(Src0, C0) - maxx(Src0 - Src1, C1),
            reference=lambda in0, in1, s0, s1, imm2: (
                np.minimum(in0.astype(np.float32), s0)
                - np.maximum(
                    in0.astype(np.float32) - in1.astype(np.float32), s1
                )
            ),
        ),
    )
    _RELU_MUL = _register(
        "ANT_RELU_MUL",
        Spec(
            body=relu(Src0) * relu(Src1),
            reference=lambda in0, in1, s0, s1, imm2: (
                np.maximum(in0.astype(np.float32), 0)
                * np.maximum(in1.astype(np.float32), 0)
            ),
        ),
    )
    _CUSTOM_REGISTERED = True


def _act_raw(nc, out_ap, in_ap, func, bias=0.0, scale=1.0):
    """InstActivation with immediate bias/scale (no const-AP conversion,
    and no bass-level Reciprocal ban)."""
    inputs = [nc.scalar.lower_ap(in_ap)]
    for arg in (bias, scale, 0.0):
        inputs.append(mybir.ImmediateValue(dtype=mybir.dt.float32, value=float(arg)))
    return nc.scalar.add_instruction(
        mybir.InstActivation(
            name=nc.get_next_instruction_name(),
            func=func,
            ins=inputs,
            outs=[nc.scalar.lower_ap(out_ap)],
        )
    )


_BUILT = None


def _build_nc():
    """Trace the single-core Bass kernel (same NEFF runs SPMD on all 8 cores)."""
    _register_custom_ops()
    nc = bacc.Bacc("TRN2", target_bir_lowering=False, debug=False)

    pred_feat = nc.dram_tensor("pred_feat", [128, NPF, NB], F32, kind="ExternalInput")
    gfeat32 = nc.dram_tensor("gfeat32", [2, M], F32, kind="ExternalInput")
    gfeat16 = nc.dram_tensor("gfeat16", [5, M], F16, kind="ExternalInput")
    idens = nc.dram_tensor("idens", [2, 128, 128], F16, kind="ExternalInput")
    pf_tail = nc.dram_tensor("pf_tail", [128, NPF], F32, kind="ExternalInput")
    gtail32 = nc.dram_tensor("gtail32", [128, 2 * MT], F32, kind="ExternalInput")
    gtail16 = nc.dram_tensor("gtail16", [128, 5 * MT], F16, kind="ExternalInput")
    out = nc.dram_tensor("out", [QSH, M], F16, kind="ExternalOutput")

    AF = mybir.ActivationFunctionType
    ALU = mybir.AluOpType

    with tile.TileContext(nc) as tc:
        with (
            tc.tile_pool(name="gpool", bufs=1) as gpool,
            tc.tile_pool(name="work3", bufs=3) as work3,
            tc.tile_pool(name="work2", bufs=3) as work2,
            tc.tile_pool(name="res", bufs=2) as res,
            tc.tile_pool(name="resa", bufs=2) as resa,
            tc.tile_pool(name="psum", bufs=8, space="PSUM") as psum_pool,
            tc.tile_pool(name="outp", bufs=3) as outp,
        ):
            # --- one-time loads: pf first (tiny, feeds everything), then the
            # nine g-row broadcasts in first-use order, spread across the SP /
            # ACT / DVE HWDGE queues so the single HWDGE device is the only
            # serializer and compute can start after ~3 rows -----------------
            pf = gpool.tile([128, NPF * NB], F32, tag="pf")

            # paired row loads: one broadcast DMA per adjacent row pair
            # (one HWDGE slot each) in first-use order, alternating queues:
            # [x1|y1] unblocks both TX_MINMAX2 customs, [cx|cy] the ACT abs,
            # [w|h] gp (and the customs' width operand), [area] s16
            def pair_load(pool_tag, dram, r0, dt, eng):
                t = gpool.tile([128, 2 * M], dt, tag=pool_tag)
                eng.dma_start(
                    t[:],
                    dram.ap()[r0 : r0 + 2, :]
                    .rearrange("r m -> (r m)")[None, :]
                    .broadcast_to([128, 2 * M]),
                )
                return t

            # f32 x1/y1 rows feed the TX_MINMAX2 customs (accuracy; same
            # cost there). The in-op x0 = x1 - w derivation and all the 4x
            # tensor_scalar ops use f16 rows. Load order = first-use order.
            HM = M // 2
            pwh = gpool.tile([128, 2 * M], F16, tag="pwh")
            gx1 = gpool.tile([128, M], F32, tag="gx1")
            gy1 = gpool.tile([128, M], F32, tag="gy1")
            # first halves of w, x1, y1 unblock vb0's first (half-width)
            # stage1 ~3us earlier; second halves + h follow
            # longest transfer first: its DMA-completion semaphore (900ns)
            # then overlaps the shorter wha/pf transfers behind it
            nc.scalar.dma_start(
                gx1[:, 0:HM], gfeat32.ap()[0:1, 0:HM].broadcast_to([128, HM])
            )
            nc.sync.dma_start(
                pwh[:, 0:HM], gfeat16.ap()[0:1, 0:HM].broadcast_to([128, HM])
            )
            nc.scalar.dma_start(pf[:], pred_feat.ap().rearrange("p a b -> p (a b)"))
            nc.sync.dma_start(
                gy1[:, 0:HM], gfeat32.ap()[1:2, 0:HM].broadcast_to([128, HM])
            )
            nc.scalar.dma_start(
                pwh[:, HM:M], gfeat16.ap()[0:1, HM:M].broadcast_to([128, M - HM])
            )
            nc.sync.dma_start(
                gx1[:, HM:M], gfeat32.ap()[0:1, HM:M].broadcast_to([128, M - HM])
            )
            nc.scalar.dma_start(
                gy1[:, HM:M], gfeat32.ap()[1:2, HM:M].broadcast_to([128, M - HM])
            )
            nc.sync.dma_start(
                pwh[:, M : 2 * M], gfeat16.ap()[1:2, :].broadcast_to([128, M])
            )
            g_area = gpool.tile([128, M], F16, tag="g_area")
            nc.sync.dma_start(
                g_area[:], gfeat16.ap()[4:5, :].broadcast_to([128, M])
            )
            # cx/cy in four half-row loads: the first halves unblock the
            # first abs ~1.2us earlier than one [cx|cy] pair DMA
            pcxy = gpool.tile([128, 2 * M], F16, tag="pcxy")
            nc.scalar.dma_start(
                pcxy[:, 0:HM], gfeat16.ap()[2:3, 0:HM].broadcast_to([128, HM])
            )
            nc.sync.dma_start(
                pcxy[:, M : M + HM],
                gfeat16.ap()[3:4, 0:HM].broadcast_to([128, HM]),
            )
            nc.scalar.dma_start(
                pcxy[:, HM:M], gfeat16.ap()[2:3, HM:M].broadcast_to([128, M - HM])
            )
            nc.sync.dma_start(
                pcxy[:, M + HM : 2 * M],
                gfeat16.ap()[3:4, HM:M].broadcast_to([128, M - HM]),
            )
            g_w = pwh[:, 0:M]
            g_h = pwh[:, M : 2 * M]
            g_w32 = g_w
            g_h32 = g_h
            g_cx = pcxy[:, 0:M]
            g_cy = pcxy[:, M : 2 * M]

            iden_sb = gpool.tile([128, 128], F16, tag="iden")
            iden_m32 = gpool.tile([128, 128], F16, tag="idenm32")

            def load_idens():
                nc.scalar.dma_start(iden_sb[:], idens.ap()[0])
                nc.sync.dma_start(iden_m32[:], idens.ap()[1])

            def pfs(row, b):
                c = row * NB + b
                return pf[:, c : c + 1]

            def pool_tt(out_ap, in0_ap, in1_ap, op, pool, tag):
                nc.gpsimd.tensor_tensor(out_ap, in0_ap, in1_ap, op=op)

            # --- software-pipelined block loop -----------------------------
            def emit_abs(b, lo=0, hi=M):
                w = hi - lo
                a_tiles = []
                for gsrc, bias_row in (
                    (g_cx, PF_B5CX), (g_cy, PF_B5CY), (g_w, PF_B5W), (g_h, PF_B5H),
                ):
                    a = resa.tile([128, M], F16, tag=f"a{bias_row}")
                    nc.scalar.activation(
                        a[:, :w], gsrc[:, lo:hi], AF.Abs, bias=pfs(bias_row, b),
                        scale=5.0,
                    )
                    a_tiles.append(a)
                return a_tiles

            def stage1(b, lo=0, hi=M, pool_ok=True):
                w = hi - lo
                # t_x | t_y packed into one pair tile
                txy = work2.tile([128, 2 * M], F16, tag="txy")
                nc.vector._custom_dve(
                    _TX_MINMAX, out=txy[:, 0:w], in0=gx1[:, lo:hi],
                    in1=g_w32[:, lo:hi],
                    s0=pfs(PF_PX1, b), s1=pfs(PF_PX0, b),
                )
                nc.vector._custom_dve(
                    _TX_MINMAX, out=txy[:, M : M + w], in0=gy1[:, lo:hi],
                    in1=g_h32[:, lo:hi],
                    s0=pfs(PF_PY1, b), s1=pfs(PF_PY0, b),
                )
                # gwpw | ghph pair
                gp = work2.tile([128, 2 * M], F16, tag="gp")
                nc.vector.tensor_scalar(
                    gp[:, 0:w], g_w[:, lo:hi], pfs(PF_PW, b), None, op0=ALU.add,
                )
                # ew and eh on Pool (tensor_tensor_reduce form)
                ee = work2.tile([128, 2 * M], F16, tag="ee")
                if pool_ok:
                    pool_tt(ee[:, 0:w], gp[:, 0:w], txy[:, 0:w],
                            ALU.subtract, work2, "ttr0")
                else:
                    nc.vector.tensor_tensor(ee[:, 0:w], gp[:, 0:w], txy[:, 0:w],
                                            op=ALU.subtract)
                # inter before gp_h/eh: Pool's union (gated by inter) starts
                # earlier; gp_h/eh only feed the later earea
                inter = work3.tile([128, M], F16, tag="inter")
                nc.vector._custom_dve(
                    _RELU_MUL, out=inter[:, :w], in0=txy[:, 0:w],
                    in1=txy[:, M : M + w],
                )
                nc.vector.tensor_scalar(
                    gp[:, M : M + w], g_h[:, lo:hi], pfs(PF_PH, b), None,
                    op0=ALU.add,
                )
                s16 = work2.tile([128, M], F16, tag="s16")
                nc.vector.tensor_scalar(
                    s16[:, :w], g_area[:, lo:hi], pfs(PF_PA, b), None, op0=ALU.add,
                )
                nc.vector.tensor_tensor(ee[:, M : M + w],
                                        gp[:, M : M + w],
                                        txy[:, M : M + w], op=ALU.subtract)
                # union | earea packed for a single paired reciprocal
                ue = work3.tile([128, 2 * M], F16, tag="ue")
                if pool_ok:
                    pool_tt(ue[:, 0:w], s16[:, :w], inter[:, :w],
                            ALU.subtract, work3, "ttr2")
                    pool_tt(ue[:, M : M + w], ee[:, 0:w], ee[:, M : M + w],
                            ALU.mult, work3, "ttr3")
                else:
                    nc.vector.tensor_tensor(ue[:, 0:w], s16[:, :w],
                                            inter[:, :w], op=ALU.subtract)
                    nc.vector.tensor_tensor(ue[:, M : M + w], ee[:, 0:w],
                                            ee[:, M : M + w], op=ALU.mult)
                return {"inter": inter, "ue": ue}

            def stage2a(b, st, lo=0, hi=M):
                w = hi - lo
                rur = res.tile([128, 2 * M], F16, tag="rur")
                _act_raw(nc, rur[:, 0:w], st["ue"][:, 0:w],
                         AF.Reciprocal, 0.0, SR)
                _act_raw(nc, rur[:, M : M + w], st["ue"][:, M : M + w],
                         AF.Reciprocal, 0.0, SR)
                st["rur"] = rur

            def stage2m(b, st, lo=0, hi=M):
                # Ap/Bp + the PE matmuls — emitted BEFORE the next stage1 so
                # PE gets the whole stage1 window to drain into psum before
                # the evacs run (kills a per-block DVE/ACT psum-wait stall)
                W = hi - lo
                rur = st["rur"]
                ue = st["ue"]
                Ap = res.tile([128, M], F16, tag="Ap")
                nc.vector.tensor_tensor(Ap[:, :W], st["inter"][:, :W],
                                        rur[:, :W], op=ALU.mult)
                Bp = res.tile([128, M], F16, tag="Bp")
                nc.vector.tensor_tensor(Bp[:, :W], ue[:, :W],
                                        rur[:, M : M + W], op=ALU.mult)

                a_tiles = st.pop("abs", None) or emit_abs(b, lo, hi)

                accs = []
                for j0 in range(0, W, 512):
                    w = min(512, W - j0)
                    acc = psum_pool.tile([128, 512], F32, tag="acc")
                    for ti, term in enumerate(a_tiles):
                        nc.tensor.matmul(
                            acc[:, :w], iden_sb[:], term[:, j0 : j0 + w],
                            start=(ti == 0), stop=False,
                        )
                    nc.tensor.matmul(
                        acc[:, :w], iden_m32[:], Ap[:, j0 : j0 + w],
                        start=False, stop=False,
                    )
                    nc.tensor.matmul(
                        acc[:, :w], iden_m32[:], Bp[:, j0 : j0 + w],
                        start=False, stop=True,
                    )
                    accs.append((acc, j0, w))
                st["accs"] = accs

            def stage2e(b, st, lo=0, hi=M, all_dve=False):
                W = hi - lo
                rows = 128
                cost = outp.tile([128, M], F16, tag="cost")
                for ci, (acc, j0, w) in enumerate(st.pop("accs")):
                    if all_dve or ci % 2 == 0:
                        nc.vector.tensor_scalar(
                            cost[:, j0 : j0 + w], acc[:, :w], 2.0, None, op0=ALU.add,
                        )
                    else:
                        nc.scalar.activation(
                            cost[:, j0 : j0 + w], acc[:, :w], AF.Copy, bias=2.0, scale=1.0,
                        )
                nc.sync.dma_start(
                    out.ap()[b * 128 : b * 128 + rows, lo:hi], cost[:rows, :W],
                )

            def stage2(b, st, lo=0, hi=M):
                stage2m(b, st, lo, hi)
                stage2e(b, st, lo, hi)

            # ---- tail: 8 preds x 1600 gt repacked as [128, 100] ----------
            gt32 = gpool.tile([128, 2 * MT], F32, tag="gt32")
            gt16 = gpool.tile([128, 5 * MT], F16, tag="gt16")
            pft = gpool.tile([128, NPF], F32, tag="pft")

            def load_tail_inputs():
                nc.scalar.dma_start(gt32[:], gtail32.ap())
                nc.sync.dma_start(gt16[:], gtail16.ap())
                nc.sync.dma_start(pft[:], pf_tail.ap())

            def tail_pass():
                W = MT
                t32 = lambda r: gt32[:, r * MT : (r + 1) * MT]
                t16 = lambda r: gt16[:, r * MT : (r + 1) * MT]
                # gt32 rows: x1, y1 ; gt16 rows: w, h, cx, cy, area
                tgx1, tgy1 = t32(0), t32(1)
                tgw, tgh, tgcx, tgcy, tga = (t16(i) for i in range(5))
                tgw32, tgh32 = tgw, tgh
                tp = lambda r: pft[:, r : r + 1]

                a_tiles = []
                for gsrc, bias_row in (
                    (tgcx, PF_B5CX), (tgcy, PF_B5CY), (tgw, PF_B5W), (tgh, PF_B5H),
                ):
                    a = gpool.tile([128, W], F16, tag=f"ta{bias_row}")
                    nc.scalar.activation(a[:], gsrc, AF.Abs, bias=tp(bias_row),
                                         scale=5.0)
                    a_tiles.append(a)
                t_x = gpool.tile([128, W], F16, tag="tt_x")
                nc.vector._custom_dve(_TX_MINMAX, out=t_x[:], in0=tgx1, in1=tgw32,
                                      s0=tp(PF_PX1), s1=tp(PF_PX0))
                t_y = gpool.tile([128, W], F16, tag="tt_y")
                nc.vector._custom_dve(_TX_MINMAX, out=t_y[:], in0=tgy1, in1=tgh32,
                                      s0=tp(PF_PY1), s1=tp(PF_PY0))
                gwpw = gpool.tile([128, W], F16, tag="tgwpw")
                nc.gpsimd.tensor_scalar(gwpw[:], tgw, tp(PF_PW), None, op0=ALU.add)
                ghph = gpool.tile([128, W], F16, tag="tghph")
                nc.gpsimd.tensor_scalar(ghph[:], tgh, tp(PF_PH), None, op0=ALU.add)
                ew = gpool.tile([128, W], F16, tag="tew")
                nc.gpsimd.tensor_tensor(ew[:], gwpw[:], t_x[:], op=ALU.subtract)
                eh = gpool.tile([128, W], F16, tag="teh")
                nc.gpsimd.tensor_tensor(eh[:], ghph[:], t_y[:], op=ALU.subtract)
                inter = gpool.tile([128, W], F16, tag="tinter")
                nc.vector._custom_dve(_RELU_MUL, out=inter[:], in0=t_x[:],
                                      in1=t_y[:])
                s16 = gpool.tile([128, W], F16, tag="ts16")
                nc.gpsimd.tensor_scalar(s16[:], tga, tp(PF_PA), None, op0=ALU.add)
                # union | earea pair for one paired reciprocal
                tue = gpool.tile([128, 2 * W], F16, tag="tue")
                nc.gpsimd.tensor_tensor(tue[:, 0:W], s16[:], inter[:],
                                        op=ALU.subtract)
                nc.gpsimd.tensor_tensor(tue[:, W : 2 * W], ew[:], eh[:],
                                        op=ALU.mult)
                trur = gpool.tile([128, 2 * W], F16, tag="trur")
                _act_raw(nc, trur[:], tue[:], AF.Reciprocal, 0.0, SR)
                Ap = gpool.tile([128, W], F16, tag="tAp")
                nc.gpsimd.tensor_tensor(Ap[:], inter[:], trur[:, 0:W],
                                        op=ALU.mult)
                Bp = gpool.tile([128, W], F16, tag="tBp")
                nc.gpsimd.tensor_tensor(Bp[:], tue[:, 0:W], trur[:, W : 2 * W],
                                        op=ALU.mult)
                acc = psum_pool.tile([128, 512], F32, tag="acc")
                for ti, term in enumerate(a_tiles):
                    nc.tensor.matmul(acc[:, :W], iden_sb[:], term[:, :W],
                                     start=(ti == 0), stop=False)
                nc.tensor.matmul(acc[:, :W], iden_m32[:], Ap[:, :W],
                                 start=False, stop=False)
                nc.tensor.matmul(acc[:, :W], iden_m32[:], Bp[:, :W],
                                 start=False, stop=True)
                tcost = gpool.tile([128, W], F16, tag="tcost")
                nc.vector.tensor_scalar(tcost[:], acc[:, :W], 2.0, None,
                                        op0=ALU.add)
                out_tail = out.ap()[NB * 128 : QSH, :].rearrange(
                    "t (s c) -> (t s) c", s=16
                )
                nc.sync.dma_start(out_tail, tcost[:])

            # virtual blocks: last full block split into column halves to
            # shorten pipeline drain
            H = M // 2
            vb = ([(0, 0, H), (0, H, M)]
                  + [(b, 0, M) for b in range(1, NB - 1)]
                  + [(NB - 1, 0, H), (NB - 1, H, M)])
            NV = len(vb)
            sts = {}

            def s2m(u):
                stage2m(vb[u][0], sts[u], *vb[u][1:])

            def s2e(u, all_dve=False):
                stage2e(vb[u][0], sts[u], *vb[u][1:], all_dve=all_dve)

            for v in range(NV):
                b, lo, hi = vb[v]
                sts[v] = stage1(b, lo, hi, pool_ok=True)
                if v == 1:
                    load_idens()
                if v == 4:
                    load_tail_inputs()
                if v - 2 >= 0:
                    s2m(v - 2)
                # evacs run one iteration behind the matmuls: PE has a whole
                # block of slack, so the evacs never wait on psum
                if v - 3 >= 0:
                    s2e(v - 3)
                if v - 1 >= 0:
                    # abs for v-1 goes on ACT *before* the recip so ACT never
                    # stalls behind Pool's earea in its in-order queue
                    pb, plo, phi = vb[v - 1]
                    sts[v - 1]["abs"] = emit_abs(pb, plo, phi)
                if v - 1 >= 0:
                    stage2a(*((vb[v - 1][0], sts[v - 1]) + vb[v - 1][1:]))
            stage2a(vb[NV - 1][0], sts[NV - 1], *vb[NV - 1][1:])
            s2m(NV - 2)
            s2e(NV - 3)
            tail_pass()
            s2m(NV - 1)
            s2e(NV - 2)
            s2e(NV - 1)

    nc.compile()
    return nc


def _host_prep(pred_boxes, gt_boxes):
    """Build per-core input maps (pure O(N+M) layout/marshaling)."""
    pred = np.asarray(pred_boxes, np.float32).reshape(N, 4)
    gt = np.asarray(gt_boxes, np.float32)

    gcx, gcy, gw, gh = gt[:, 0], gt[:, 1], gt[:, 2], gt[:, 3]
    gx0 = gcx - np.float32(0.5) * gw
    gx1 = gcx + np.float32(0.5) * gw
    gy0 = gcy - np.float32(0.5) * gh
    gy1 = gcy + np.float32(0.5) * gh
    garea = (gx1 - gx0) * (gy1 - gy0)
    gfeat32 = np.stack([gx1, gy1]).astype(np.float32)
    gfeat16 = np.stack([gw, gh, gcx, gcy, garea]).astype(np.float16)
    idens = np.stack(
        [np.eye(128), np.eye(128) * (-2.0 * SR)]
    ).astype(np.float16)

    # tail g-rows repacked to [128 = 8 preds x 16 slices, R*MT]
    def tail_rows(rows, dt):
        r = np.stack(rows)                       # [R, 1600]
        sl = r.reshape(len(rows), 16, MT)        # [R, 16, MT]
        per_slice = sl.transpose(1, 0, 2).reshape(16, len(rows) * MT)
        return np.tile(per_slice, (8, 1)).astype(dt)

    gtail32 = tail_rows([gx1, gy1], np.float32)
    gtail16 = tail_rows([gw, gh, gcx, gcy, garea], np.float16)

    def feats(arr):
        pcx, pcy, pw, ph = (arr[..., k] for k in range(4))
        px0 = pcx - np.float32(0.5) * pw
        px1 = pcx + np.float32(0.5) * pw
        py0 = pcy - np.float32(0.5) * ph
        py1 = pcy + np.float32(0.5) * ph
        pa = (px1 - px0) * (py1 - py0)
        return np.stack(
            [px0, px1, py0, py1, pw, ph, pa,
             -5.0 * pcx, -5.0 * pcy, -5.0 * pw, -5.0 * ph],
            axis=-2,
        ).astype(np.float32)

    in_maps = []
    for c in range(NCORES):
        sl = pred[c * QSH : (c + 1) * QSH]
        blocks = sl[: NB * 128].reshape(NB, 128, 4).transpose(1, 0, 2)
        pf = feats(blocks)                       # [128, NPF, NB]
        tail_rep = np.repeat(sl[NB * 128 :], 16, axis=0)   # [128, 4]
        pft = feats(tail_rep[:, None, :])[:, :, 0]         # [128, NPF]
        in_maps.append(
            {"pred_feat": pf, "gfeat32": gfeat32, "gfeat16": gfeat16,
             "idens": idens, "pf_tail": pft,
             "gtail32": gtail32, "gtail16": gtail16}
        )
    return in_maps


def _get_nc():
    global _BUILT
    if _BUILT is None:
        _BUILT = _build_nc()
    return _BUILT


def kernel(pred_boxes, gt_boxes):
    nc = _get_nc()
    in_maps = _host_prep(pred_boxes, gt_boxes)
    res = run_bass_kernel_spmd(nc, in_maps, list(range(NCORES)))
    slabs = [res.results[c]["out"] for c in range(NCORES)]
    return np.concatenate(slabs, axis=0).reshape(B, Q, M).astype(np.float32)
